# revision 1
# baseline (speedup 1.0000x reference)
"""Multi-head self-attention (B=8, S=1024, D=768, H=12) on 8 trn2 cores.

Sharding: data-parallel over batch - core b computes attention for Q[b].
No collectives. All inputs are host-prepacked into exact SBUF layouts
(>=512B contiguous runs per DMA descriptor); output is written in
natural [S, D] layout as bf16 and upcast on the host.

Per-core dataflow:
  - q^T/k^T = (W X^T + b) via PE matmuls (d_in on partitions), bias
    added on DVE while copying PSUM->SBUF bf16.
  - v packed [128, sc, 12*65]: 64 v-columns + a ones column per head,
    so the ctx matmul also emits the softmax denominator Z.
  - scores stay transposed ([s_k part, s_q free]); exp on ACT with the
    1/sqrt(dk) scale fused. The score/exp stream is FLAT over 192
    chunks (12 heads x 16): PSUM tiles [128,3,512] (3 banks, 2 bufs)
    fill chunk-by-chunk and are exp'd whole, so ACT instruction
    boundaries ignore head boundaries (fewer, larger ACT ops).
  - ctx runs NON-transposed, out[q, d], M=128 q rows per accumulation
    group: lhsT = exp tile [k part, q], rhs = packed v [k part, 65].
    This halves PE cycles vs the ctx^T form and makes 1/Z a
    per-partition scalar (DVE reciprocal + tensor_scalar_mul).
    The 4 groups of a half-head share ONE PSUM bank: only the very
    first matmul uses start=True (zeroes the bank's pending-zero
    region); later groups accumulate into pending-zero bytes, which
    assign. Matmuls are kc-major so only the last 4 wait on the
    final exp.

Scheduling: emission order == Tile scheduler priority. Each exp-group
"slot" carries the 3 score chunks plus one filler unit (proj block /
v block / ctx quad / output wave), deadline-ordered so PE production
rate-matches ACT's drain rate (psSc is 2 tiles deep and PE executes
in-order). A short warm-up dummy-matmul chain keeps the PE p-state
ramp warm through the initial DMA fill. gpsimd only touches SBUF
(PSUM is not accessible to it on HW).

Projections run in fp8 (e4m3) hi/lo split form: X and W are each
pre-scaled (x8 / x64, keeping residuals out of fp8's subnormal range)
and split on the host into fp8 hi + fp8 residual; three DoubleRow
matmul terms (hi*hi + lo*hi + hi*lo) recover better-than-bf16 accuracy
at 0.75x the bf16 PE cost (DoubleRow folds 2 contraction k-tiles per
pass at 0.5 cyc/row). The 1/512 descale is fused into the bias add.
Scores and ctx stay bf16.

The score stream is permuted so the first 8 chunks depend only on the
first X half + pair-0 projections (heads 0 and 1 both live in pair 0),
hiding the second X-half DMA behind the first exps. Normalization uses
one reciprocal + one broadcast tensor_mul (stride-0 AP on the 1/Z
column) per ctx quad.

The startup-critical inputs (first X half + pair-0 W slices, hi and
lo) are host-packed into one contiguous blob per half so the first
projections are fed by two large penalty-free DMAs.

The startup K projection's bias-add runs on ACT (Identity activation,
AP bias, fused descale) in parallel with the Q add on DVE, since ACT
is idle until the first exp.

Engine budget (cost model): PE ~99us busy (proj 35 + scores 41 +
ctx 21 + p-state ramp), ACT ~94us (exp floor; the critical chain:
first exp ~9.7us after the DMA fill, gapless to ~105.2us), DVE ~35us,
Pool ~15us. Close-out ~4.5us (final ctx quad + normalize + output DMA
semaphore + drain ladder). Timeline ~109.9us; PE and ACT paths are
co-critical within ~1us.
"""

import ml_dtypes
import numpy as np

import concourse.bass as bass
import concourse.mybir as mybir
import concourse.tile as tile
from concourse.bass_utils import run_bass_kernel_spmd

F32 = mybir.dt.float32
BF16 = mybir.dt.bfloat16
FP8 = mybir.dt.float8e4
DR = mybir.MatmulPerfMode.DoubleRow

S = 1024
D = 768
H = 12
DK = 64
KC = D // 128     # 6 contraction chunks
MC = D // 128     # 6 head pairs
SC = S // 128     # 8 sequence chunks
NSQ = S // 512    # 2 query-column blocks
SCALE = 1.0 / np.sqrt(DK)
VROW = DK + 1     # 64 v columns + ones column per head
CPH = NSQ * SC    # 16 score chunks per head
NCH = H * CPH     # 192 score chunks total
GG = 3            # chunks per exp group
NSINGLE = 0       # no singles
EXP_BUFS = 28
XSC = 8.0          # fp8 pre-scale for X (hi/lo split)
WSC = 64.0         # fp8 pre-scale for W
DESC = 1.0 / (XSC * WSC)

# score-chunk stream order: front-load the 8 chunks that depend only on
# the first X half + pair-0 projections (heads 0 AND 1 share pair 0), so
# the exp stream never stalls on the second X-half DMA.
_STREAM = [(0, 0, 0), (0, 0, 1), (0, 0, 2), (0, 0, 3),
           (1, 0, 0), (1, 0, 1), (1, 0, 2), (1, 0, 3)]
for _h in range(H):
    for _j in range(NSQ):
        for _kc in range(SC):
            if (_h, _j, _kc) not in _STREAM[:8]:
                _STREAM.append((_h, _j, _kc))
assert len(_STREAM) == NCH

# (h, j, kc) -> (group index, pos); groups of GG over the stream order
_CHUNK_GRP = {}
_GROUPS = []      # list of (start pos, length)
for _g in range(NCH // GG):
    _GROUPS.append((_g * GG, GG))
    for _i in range(GG):
        _CHUNK_GRP[_STREAM[_g * GG + _i]] = (_g, _i)
NSLOT = len(_GROUPS)   # 64


def _split_excess_waits(nc, max_waits=1):
    """walrus encodes at most one sem-wait per instruction; spread extra
    waits onto EventSemaphore instructions."""
    for fn in nc.m.functions:
        for bb in fn.blocks:
            out = []
            for ins in bb.instructions:
                si = getattr(ins, "sync_info", None)
                ow = list(si.on_wait) if (si is not None and si.on_wait) else []
                if len(ow) > max_waits:
                    head, tail = ow[:-max_waits], ow[-max_waits:]
                    for j in range(0, len(head), max_waits):
                        ev = mybir.InstEventSemaphore(
                            name=f"evsplit-{ins.name}-{j}", ins=[], outs=[])
                        ev.engine = ins.engine
                        ev.sync_info = mybir.SyncInfo(
                            on_wait=head[j:j + max_waits], on_update=[])
                        out.append(ev)
                    ins.sync_info = mybir.SyncInfo(
                        on_wait=tail, on_update=list(si.on_update))
                out.append(ins)
            bb.instructions = out


def build_nc():
    nc = bass.Bass(trn_type="TRN2")

    # host-prepacked inputs (exact SBUF layouts)
    qth = nc.dram_tensor("qth", [2, NSQ, 128, KC, 512], FP8,
                         kind="ExternalInput").ap()
    blob0 = nc.dram_tensor("blob0", [2, 128, 4608], FP8,
                           kind="ExternalInput").ap()
    wqh = nc.dram_tensor("wqh", [2, 128, KC, D], FP8,
                         kind="ExternalInput").ap()
    wkh = nc.dram_tensor("wkh", [2, 128, KC, D], FP8,
                         kind="ExternalInput").ap()
    wvh = nc.dram_tensor("wvh", [2, 128, KC, D], FP8,
                         kind="ExternalInput").ap()
    bqh = nc.dram_tensor("bqh", [128, MC], F32, kind="ExternalInput").ap()
    bkh = nc.dram_tensor("bkh", [128, MC], F32, kind="ExternalInput").ap()
    bvbh = nc.dram_tensor("bvbh", [128, H, DK], BF16,
                          kind="ExternalInput").ap()
    ctxo = nc.dram_tensor("ctxo", [S, D], BF16, kind="ExternalOutput").ap()

    with tile.TileContext(nc) as tc:
        with (
            tc.tile_pool(name="singles", bufs=1) as singles,
            tc.tile_pool(name="psSc", bufs=2, space="PSUM") as psSc,
            tc.tile_pool(name="psP", bufs=2, space="PSUM") as psP,
            tc.tile_pool(name="expp", bufs=EXP_BUFS) as expp,
            tc.tile_pool(name="recp", bufs=6) as recp,
        ):
            # ---- persistent SBUF arrays --------------------------------
            # blob0: X^T cols 0:512 + pair-0 W slices, one DMA per hi/lo
            blob0_sb = [singles.tile([128, 4608], FP8, name=f"blob0_{a}")
                        for a in range(2)]
            qt0_sb = [blob0_sb[a][:, 0:3072]
                      .rearrange("p (c s) -> p c s", s=512)
                      for a in range(2)]               # X^T cols 0:512 hi/lo
            wq0_sb = [blob0_sb[a][:, 3072:3840]
                      .rearrange("p (c n) -> p c n", n=128)
                      for a in range(2)]               # Wq^T pair 0
            wk0_sb = [blob0_sb[a][:, 3840:4608]
                      .rearrange("p (c n) -> p c n", n=128)
                      for a in range(2)]
            qt1_sb = [singles.tile([128, KC, 512], FP8, name=f"qt1_{a}")
                      for a in range(2)]
            wq1_sb = [singles.tile([128, KC, 128], FP8, name=f"wq1_{a}")
                      for a in range(2)]               # Wq^T pair 1
            wqB_sb = [singles.tile([128, KC, 512], FP8, name=f"wqB_{a}")
                      for a in range(2)]
            wk1_sb = [singles.tile([128, KC, 128], FP8, name=f"wk1_{a}")
                      for a in range(2)]
            wkB_sb = [singles.tile([128, KC, 512], FP8, name=f"wkB_{a}")
                      for a in range(2)]
            wv_sb = [singles.tile([128, KC, D], FP8, name=f"wv_{a}")
                     for a in range(2)]
            qT_sb = singles.tile([128, MC, S], BF16)      # q^T (d_out, s)
            kT_sb = singles.tile([128, MC, S], BF16)
            v_sb = singles.tile([128, SC, H * VROW], BF16)
            bq_sb = singles.tile([128, MC], F32)
            bk_sb = singles.tile([128, MC], F32)
            bvb_sb = singles.tile([128, H, DK], BF16)     # bv bcast over parts
            cto_sb = singles.tile([128, SC, D], BF16)     # ctx staging

            # ---- input DMAs (order == DMA-device service order) --------
            # critical path: bqh/bkh (tiny) -> wq first cols -> qt half 0
            # -> wk first cols -> qt half 1 -> bvbh -> wv -> weight tails
            nc.sync.dma_start(out=blob0_sb[0], in_=blob0[0])
            nc.sync.dma_start(out=blob0_sb[1], in_=blob0[1])
            nc.sync.dma_start(out=bq_sb, in_=bqh)
            nc.sync.dma_start(out=bk_sb, in_=bkh)
            nc.sync.dma_start(out=qt1_sb[0], in_=qth[0][1])
            nc.sync.dma_start(out=qt1_sb[1], in_=qth[1][1])
            nc.sync.dma_start(out=wq1_sb[0], in_=wqh[0][:, :, 128:256])
            nc.sync.dma_start(out=wk1_sb[0], in_=wkh[0][:, :, 128:256])
            nc.sync.dma_start(out=wq1_sb[1], in_=wqh[1][:, :, 128:256])
            nc.sync.dma_start(out=wk1_sb[1], in_=wkh[1][:, :, 128:256])
            nc.sync.dma_start(out=bvb_sb, in_=bvbh)
            nc.sync.dma_start(out=wv_sb[0], in_=wvh[0])
            nc.sync.dma_start(out=wv_sb[1], in_=wvh[1])
            nc.sync.dma_start(out=wqB_sb[0], in_=wqh[0][:, :, 256:768])
            nc.sync.dma_start(out=wkB_sb[0], in_=wkh[0][:, :, 256:768])
            nc.sync.dma_start(out=wqB_sb[1], in_=wqh[1][:, :, 256:768])
            nc.sync.dma_start(out=wkB_sb[1], in_=wkh[1][:, :, 256:768])

            # ones columns of v (col 64 of each 65-wide head group)
            v4 = v_sb.rearrange("p s (h c) -> p s h c", c=VROW)
            nc.vector.memset(v4[:, :, :, DK:DK + 1], 1.0)

            # PE warm-up: keep the tensor engine busy during the input DMA
            # fill so the p-state ramp (3us to full clock) completes before
            # real matmuls start.
            wrm = singles.tile([1, 512], BF16)
            nc.vector.memset(wrm, 0.0)
            for wi in range(6):
                wps = psP.tile([1, 512], F32, tag="p", name=f"warm_{wi}")
                nc.tensor.matmul(wps, lhsT=wrm[:, 0:1], rhs=wrm,
                                 start=True, stop=True)
            for wi in range(6):
                wps = psP.tile([1, 512], F32, tag="p", name=f"warmb_{wi}")
                nc.tensor.matmul(wps[:, 0:128], lhsT=wq0_sb[0][0:1, 0, 0:1],
                                 rhs=wq0_sb[0][0:1, 0, 0:128],
                                 start=True, stop=True)

            # ---- emission helpers (emission order == scheduler priority)

            TERMS = ((0, 0), (1, 0), (0, 1))   # (x hi/lo, w hi/lo)

            def proj_qk(mc, which, n, act_add=False):
                """One [128,512] block of q^T (which=0) / k^T (which=1).
                hi/lo fp8 split: 3 DoubleRow terms (hi*hi, lo*hi, hi*lo)
                recover ~bf16 accuracy at 0.75x the bf16 PE cost."""
                w0, w1, wB, b_sb, o_sb = (
                    (wq0_sb, wq1_sb, wqB_sb, bq_sb, qT_sb) if which == 0
                    else (wk0_sb, wk1_sb, wkB_sb, bk_sb, kT_sb))
                w_sb, mco = ((w0, 0) if mc == 0 else
                             (w1, 0) if mc == 1 else (wB, mc - 2))
                ps = psP.tile([128, 512], F32, tag="p",
                              name=f"pj_{mc}_{which}_{n}")
                qt_n = qt0_sb if n == 0 else qt1_sb
                for ti, (xa, wa) in enumerate(TERMS):
                    for kt in range(KC // 2):
                        nc.tensor.matmul(
                            ps,
                            lhsT=w_sb[wa][:, 2 * kt:2 * kt + 2,
                                          mco * 128:(mco + 1) * 128],
                            rhs=qt_n[xa][:, 2 * kt:2 * kt + 2, :],
                            start=(ti == 0 and kt == 0),
                            stop=(ti == 2 and kt == KC // 2 - 1),
                            perf_mode=DR,
                        )
                if act_add:
                    # ACT is idle before the first exp; Identity with an AP
                    # bias and the descale fused runs this add in parallel
                    # with the DVE add of the sibling projection.
                    nc.scalar.activation(
                        out=o_sb[:, mc, n * 512:(n + 1) * 512],
                        in_=ps,
                        func=mybir.ActivationFunctionType.Identity,
                        bias=b_sb[:, mc:mc + 1],
                        scale=DESC,
                    )
                else:
                    nc.vector.tensor_scalar(
                        out=o_sb[:, mc, n * 512:(n + 1) * 512],
                        in0=ps,
                        scalar1=DESC,
                        scalar2=b_sb[:, mc:mc + 1],
                        op0=mybir.AluOpType.mult,
                        op1=mybir.AluOpType.add,
                    )

            def proj_v(sc, half):
                """v[s, d] block: seq chunk sc, d_out half (6 heads)."""
                ps = psP.tile([128, 512], F32, tag="p", name=f"pv_{sc}_{half}")
                if sc < 4:
                    qt_n, scl = qt0_sb, sc
                else:
                    qt_n, scl = qt1_sb, sc - 4
                for ti, (xa, wa) in enumerate(TERMS):
                    for kt in range(KC // 2):
                        nc.tensor.matmul(
                            ps[:, 0:384],
                            lhsT=qt_n[xa][:, 2 * kt:2 * kt + 2,
                                          scl * 128:(scl + 1) * 128],
                            rhs=wv_sb[wa][:, 2 * kt:2 * kt + 2,
                                          half * 384:(half + 1) * 384],
                            start=(ti == 0 and kt == 0),
                            stop=(ti == 2 and kt == KC // 2 - 1),
                            perf_mode=DR,
                        )
                nc.vector.scalar_tensor_tensor(
                    out=v4[:, sc, 6 * half:6 * half + 6, 0:DK],
                    in0=ps[:, 0:384].rearrange("p (h c) -> p h c", c=DK),
                    scalar=DESC,
                    in1=bvb_sb[:, 6 * half:6 * half + 6, :],
                    op0=mybir.AluOpType.mult,
                    op1=mybir.AluOpType.add,
                )

            exps = [None] * NSLOT

            def emit_group(g):
                """Emit score chunks of exp group g, then the exp."""
                g0, glen = _GROUPS[g]
                ps = psSc.tile([128, GG, 512], F32, tag="sc", name=f"sc_{g}")
                for i in range(glen):
                    h, j, kc = _STREAM[g0 + i]
                    mc, pb = h // 2, (h % 2) * DK
                    nc.tensor.matmul(
                        ps[:, i, :],
                        lhsT=kT_sb[pb:pb + DK, mc, kc * 128:(kc + 1) * 128],
                        rhs=qT_sb[pb:pb + DK, mc, j * 512:(j + 1) * 512],
                        start=True, stop=True,
                    )
                et = expp.tile([128, GG, 512], BF16, tag="exp", name=f"ex_{g}")
                nc.scalar.activation(
                    out=et[:, 0:glen, :],
                    in_=ps[:, 0:glen, :],
                    func=mybir.ActivationFunctionType.Exp,
                    scale=float(SCALE),
                )
                exps[g] = et

            def ctx_quad(h, half, last=False):
                """ctx[q, :] + Z for head h, q-chunks 4*half..4*half+3.
                All 4 accumulation groups share ONE psum bank: only the
                very first matmul uses start=True (zeroes the bank's
                pending-zero region); later groups accumulate into
                pending-zero bytes, which assign."""
                ps = psP.tile([128, 4 * VROW], F32, tag="p",
                              name=f"cx_{h}_{half}")
                ps4 = ps.rearrange("p (q c) -> p q c", c=VROW)
                j = half
                for kc in range(SC):
                    t, i = _CHUNK_GRP[(h, j, kc)]
                    for q in range(4):
                        qc = 4 * half + q
                        qcl = qc % 4
                        nc.tensor.matmul(
                            ps4[:, q, :],
                            lhsT=exps[t][:, i, qcl * 128:(qcl + 1) * 128],
                            rhs=v_sb[:, kc, h * VROW:(h + 1) * VROW],
                            start=(kc == 0 and q == 0),
                            stop=(kc == SC - 1 and q == 3),
                            skip_group_check=True,
                        )
                if last:
                    cxs = ps4      # skip the copy; psP pressure is over
                else:
                    cxs = recp.tile([128, 4, VROW], F32, tag="cxs",
                                    name=f"cxs_{h}_{half}")
                    nc.vector.tensor_copy(out=cxs, in_=ps4)
                rec = recp.tile([128, 4], F32, tag="rec",
                                name=f"rec_{h}_{half}")
                nc.vector.reciprocal(
                    out=rec.rearrange("p (q o) -> p q o", o=1),
                    in_=cxs[:, :, DK:DK + 1])
                # one multiply for all 4 q-chunks: broadcast 1/Z along the
                # d dim via a stride-0 AP
                recb = bass.AP(tensor=rec.tensor, offset=rec.offset,
                               ap=[rec.ap[0], [rec.ap[1][0], 4], [0, DK]])
                muleng = nc.vector if last else nc.gpsimd
                muleng.tensor_mul(
                    out=cto_sb[:, 4 * half:4 * half + 4,
                               h * DK:(h + 1) * DK],
                    in0=cxs[:, :, 0:DK],
                    in1=recb,
                )

            ctxor = ctxo.rearrange("(qc p) d -> p qc d", p=128)

            def out_wave(c0, c1):
                """DMA output cols [c0:c1) for all 8 q-chunks (one DMA)."""
                nc.sync.dma_start(out=ctxor[:, :, c0:c1],
                                  in_=cto_sb[:, :, c0:c1])

            def out_wave_half(c0, c1, hf):
                nc.sync.dma_start(out=ctxor[:, 4 * hf:4 * hf + 4, c0:c1],
                                  in_=cto_sb[:, 4 * hf:4 * hf + 4, c0:c1])

            # ---- slot schedule ----------------------------------------
            # per slot: 3 score chunks, then the listed units. proj units
            # are deadline-placed (their qT/kT rows feed later chunks);
            # ctx quads sit on even slots so the DVE normalize latency
            # hides under the following odd slot's work.
            plan = {}

            def put(slot, unit):
                plan.setdefault(slot, []).append(unit)

            for i, u in enumerate([("K", 0, 1), ("Q", 0, 1), ("Q", 1, 0),
                                   ("K", 1, 0), ("K", 1, 1), ("Q", 1, 1)]):
                put(i, u)
            for sc in range(SC):
                put(6 + sc, ("V", sc, 0))
            for i, u in enumerate([("Q", 2, 0), ("K", 2, 0), ("K", 2, 1),
                                   ("Q", 2, 1)]):
                put(14 + i, u)
            seq = [("Q", 3, 0), ("CX", 0, 0), ("K", 3, 0), ("CX", 0, 1),
                   ("K", 3, 1), ("CX", 1, 0), ("Q", 3, 1), ("CX", 1, 1),
                   ("Q", 4, 0), ("V", 0, 1), ("K", 4, 0), ("V", 1, 1),
                   ("K", 4, 1), ("V", 2, 1), ("Q", 4, 1), ("V", 3, 1),
                   ("V", 4, 1), ("V", 5, 1), ("V", 6, 1), ("V", 7, 1),
                   ("CX", 2, 0), ("CX", 2, 1), ("CX", 3, 0), ("CX", 3, 1),
                   ("Q", 5, 0), ("CX", 4, 0), ("K", 5, 0), ("CX", 4, 1),
                   ("K", 5, 1), ("CX", 5, 0), ("Q", 5, 1), ("CX", 5, 1),
                   ("CX", 6, 0), ("CX", 6, 1), ("CX", 7, 0), ("CX", 7, 1),
                   ("CX", 8, 0), ("CX", 8, 1), ("CX", 9, 0), ("CX", 9, 1),
                   ("CX", 10, 0), ("CX", 10, 1)]
            for i, u in enumerate(seq):
                put(18 + i, u)                   # slots 18..61
            put(61, ("CX", 11, 0))
            put(62, ("WH", (H - 1) * DK, H * DK, 0))
            put(40, ("W", 0 * DK, 3 * DK))       # heads 0-2 done @39
            put(50, ("W", 3 * DK, 6 * DK))       # heads 3-5 done @49
            put(56, ("W", 6 * DK, 9 * DK))       # heads 6-8 done @55
            put(60, ("W", 9 * DK, 11 * DK))      # heads 9-10 done @59

            # ---- software pipeline ------------------------------------
            proj_qk(0, 0, 0)
            proj_qk(0, 1, 0, act_add=True)
            for g in range(NSLOT):
                emit_group(g)
                for u in plan.get(g, ()):
                    if u[0] == "Q":
                        proj_qk(u[1], 0, u[2])
                    elif u[0] == "K":
                        proj_qk(u[1], 1, u[2])
                    elif u[0] == "V":
                        proj_v(u[1], u[2])
                    elif u[0] == "CX":
                        ctx_quad(u[1], u[2])
                    elif u[0] == "WH":
                        out_wave_half(u[1], u[2], u[3])
                    elif u[0] == "D":
                        for di in range(u[1]):
                            wps = psP.tile([1, 512], F32, tag="p",
                                           name=f"dummy_{g}_{di}")
                            nc.tensor.matmul(
                                wps[:, 0:384], lhsT=wrm[:, 0:1],
                                rhs=wrm[:, 0:384], start=True, stop=True)
                    else:
                        out_wave(u[1], u[2])
            ctx_quad(H - 1, 1, last=True)
            out_wave_half((H - 1) * DK, H * DK, 1)

    _split_excess_waits(nc)
    return nc


_NC_CACHE = None


def _get_nc():
    global _NC_CACHE
    if _NC_CACHE is None:
        _NC_CACHE = build_nc()
    return _NC_CACHE


def kernel(Q, Wq, bq, Wk, bk, Wv, bv):
    BF = ml_dtypes.bfloat16
    F8 = ml_dtypes.float8_e4m3
    Q = np.asarray(Q, np.float32)

    def hilo(a, scale):
        # fp8 hi + fp8 residual of pre-scaled values; hi*hi + lo*hi +
        # hi*lo recovers better-than-bf16 accuracy. Pre-scaling keeps
        # small W values and the residuals out of fp8's subnormal range.
        a = a * scale
        hi = a.astype(F8)
        lo = (a - hi.astype(np.float32)).astype(F8)
        return np.ascontiguousarray(np.stack([hi, lo]))

    def packw(W):
        # [128, KC, D]: [p, c, n] = W^T[c*128+p, n] = W[n, c*128+p]
        return hilo(np.asarray(W, np.float32).T.reshape(KC, 128, D)
                    .transpose(1, 0, 2), WSC)

    def packb(b):
        return np.ascontiguousarray(
            np.asarray(b, np.float32).reshape(MC, 128).T)

    wqh, wkh, wvh = packw(Wq), packw(Wk), packw(Wv)
    bqh, bkh = packb(bq), packb(bk)
    bvbh = np.ascontiguousarray(np.broadcast_to(
        np.asarray(bv, np.float32).reshape(1, H, DK).astype(BF),
        (128, H, DK)))

    nc = _get_nc()
    in_maps = []
    for b_i in range(Q.shape[0]):
        # qth [2, NSQ, 128, 6, 512]: [a, n, p, c, s] = Q[b][n*512+s, c*128+p]
        qth = hilo(Q[b_i].reshape(NSQ, 512, KC, 128).transpose(0, 3, 2, 1),
                   XSC)
        blob0 = np.ascontiguousarray(np.concatenate(
            [qth[:, 0].reshape(2, 128, 3072),
             wqh[:, :, :, 0:128].reshape(2, 128, 768),
             wkh[:, :, :, 0:128].reshape(2, 128, 768)], axis=2))
        in_maps.append({
            "qth": qth, "blob0": blob0,
            "wqh": wqh, "wkh": wkh, "wvh": wvh,
            "bqh": bqh, "bkh": bkh, "bvbh": bvbh,
        })
    res = run_bass_kernel_spmd(nc, in_maps, core_ids=list(range(len(in_maps))))
    out = np.stack([np.asarray(r["ctxo"], np.float32) for r in res.results])
    return out

